# revision 1
# baseline (speedup 1.0000x reference)
"""GNN message-passing (MPNN w/ LSTM update + gated sum pooling) on 8 trn2 cores.

Strategy:
  - Edges partitioned by dst node range across 8 cores (12500 nodes/core).
  - Within a core, edges grouped by 128-node "window" of their dst; each
    window padded to a common edge count W_E (multiple of 128).
  - Message MLP factored: pre = u[dst] + v[src] + ea @ W1c.T, where
    u = h @ W1a.T + b1 and v = h @ W1b.T are per-node projections
    (computed on device each step, stored in DRAM, gathered per edge
    via indirect DMA).
  - Scatter-add via one-hot matmul accumulated in PSUM per window, in
    transposed layout: zaggT[d, s] += z.T @ B  (lhsT=z, rhs=B), then
    aT = W2 @ zaggT + b2 (x) deg  -- no on-chip transposes anywhere.
  - LSTM + readout fused into the per-window loop; node state kept as
    hT [64, nodes] (features on partitions).
  - 2 launches of ONE step-generic NEFF; host exchanges h between steps
    and sums the 8 per-core readout partials.
"""

import os

import numpy as np
import ml_dtypes

import concourse.bass as bass
import concourse.mybir as mybir
import concourse.tile as tile
from concourse.bass_utils import run_bass_kernel_spmd

BF16 = ml_dtypes.bfloat16
FP32 = np.float32

# problem sizes (hardcoded per spec)
N = 100000
E = 1600000
D = 64
DE = 32
G = 50
P_STEPS = 2
CORES = 8

WIN = 128
SUB = 128
CHUNK = 1024
KSUB = CHUNK // SUB  # 8

LAST_EXEC_NS = None  # set when tracing enabled


def _cfg(n, e, cores):
    npc = n // cores
    nwin = (npc + WIN - 1) // WIN
    return dict(N=n, E=e, CORES=cores, NPC=npc, NWIN=nwin,
                NPC_PAD=nwin * WIN, NFULL=cores * nwin * WIN)


# ----------------------------------------------------------------------------
# device kernel builder (one message-passing step, SPMD over cores)
# ----------------------------------------------------------------------------

def _build(cfg, spw, nchunk, epa):
    """Build the step NEFF. spw = subtiles per window, epa = padded edges."""
    NWIN = cfg["NWIN"]
    NPC_PAD = cfg["NPC_PAD"]
    NFULL = cfg["NFULL"]
    NPC = cfg["NPC"]
    nsub = NWIN * spw  # real subtiles
    nwin_full = NFULL // WIN  # windows across all cores (v projection)

    fp = mybir.dt.float32
    bf = mybir.dt.bfloat16
    i32 = mybir.dt.int32

    nc = bass.Bass("TRN2", target_bir_lowering=False, debug=False)

    # --- I/O -----------------------------------------------------------------
    hT_full = nc.dram_tensor("hT_full", [D + 1, NFULL], bf, kind="ExternalInput")
    hT_loc = nc.dram_tensor("hT_loc", [D + 1, NPC_PAD], bf, kind="ExternalInput")
    cT_in = nc.dram_tensor("cT_in", [D, NPC_PAD], fp, kind="ExternalInput")
    eaT = nc.dram_tensor("eaT", [DE, epa], bf, kind="ExternalInput")
    idx_il = nc.dram_tensor("idx_il", [nchunk, SUB, 2 * KSUB], i32, kind="ExternalInput")
    slot_il = nc.dram_tensor("slot_il", [nchunk, SUB, KSUB], fp, kind="ExternalInput")
    deg_in = nc.dram_tensor("deg_in", [1, NPC_PAD], bf, kind="ExternalInput")
    w1aT = nc.dram_tensor("w1aT", [D + 1, D], bf, kind="ExternalInput")
    w1bT = nc.dram_tensor("w1bT", [D + 1, D], bf, kind="ExternalInput")
    w1cT = nc.dram_tensor("w1cT", [DE, D], bf, kind="ExternalInput")
    w2T = nc.dram_tensor("w2T", [D, D], bf, kind="ExternalInput")
    b2row = nc.dram_tensor("b2row", [1, D], bf, kind="ExternalInput")
    wihT = nc.dram_tensor("wihT", [D, 4 * D], bf, kind="ExternalInput")
    whhT = nc.dram_tensor("whhT", [D, 4 * D], bf, kind="ExternalInput")
    biasg = nc.dram_tensor("biasg", [D, 4], fp, kind="ExternalInput")
    gmT = nc.dram_tensor("gmT", [D + 1, G], bf, kind="ExternalInput")
    fmT = nc.dram_tensor("fmT", [D + 1, G], bf, kind="ExternalInput")

    hT_out = nc.dram_tensor("hT_out", [D, NPC_PAD], bf, kind="ExternalOutput")
    cT_out = nc.dram_tensor("cT_out", [D, NPC_PAD], fp, kind="ExternalOutput")
    partial = nc.dram_tensor("partial", [1, G], fp, kind="ExternalOutput")

    # internal scratch
    u_dram = nc.dram_tensor("u_dram", [NPC_PAD, D], bf)
    v_dram = nc.dram_tensor("v_dram", [NFULL, D], bf)

    AF = mybir.ActivationFunctionType
    gate_funcs = [AF.Sigmoid, AF.Sigmoid, AF.Tanh, AF.Sigmoid]  # i, f, g, o

    with tile.TileContext(nc) as tc:
        with (
            tc.tile_pool(name="const", bufs=1) as cp,
            tc.tile_pool(name="proj", bufs=2) as pp,
            tc.tile_pool(name="edge", bufs=3) as ep,
            tc.tile_pool(name="winp", bufs=2) as wp,
            tc.tile_pool(name="psum", bufs=2, space="PSUM") as ps,
            tc.tile_pool(name="psum3", bufs=3, space="PSUM") as ps3,
            tc.tile_pool(name="psumw", bufs=3, space="PSUM") as psw,
        ):
            # --- constants in SBUF -------------------------------------------
            def load_const(t, shape, dtype):
                s = cp.tile(shape, dtype, tag=t.name)
                nc.sync.dma_start(out=s[:], in_=t[:])
                return s

            w1aT_s = load_const(w1aT, [D + 1, D], bf)
            w1bT_s = load_const(w1bT, [D + 1, D], bf)
            w1cT_s = load_const(w1cT, [DE, D], bf)
            w2T_s = load_const(w2T, [D, D], bf)
            b2row_s = load_const(b2row, [1, D], bf)
            wihT_s = load_const(wihT, [D, 4 * D], bf)
            whhT_s = load_const(whhT, [D, 4 * D], bf)
            biasg_s = load_const(biasg, [D, 4], fp)
            gmT_s = load_const(gmT, [D + 1, G], bf)
            fmT_s = load_const(fmT, [D + 1, G], bf)
            deg_s = load_const(deg_in, [1, NPC_PAD], bf)

            iota_i = cp.tile([SUB, SUB], i32, tag="iota_i")
            nc.gpsimd.iota(iota_i[:], pattern=[[1, SUB]], base=0, channel_multiplier=0)
            iota_f = cp.tile([SUB, SUB], fp, tag="iota_f")
            nc.vector.tensor_copy(iota_f[:], iota_i[:])

            ones_col = cp.tile([SUB, 1], fp, tag="ones_col")
            nc.vector.memset(ones_col[:], 1.0)

            acc = cp.tile([SUB, G], fp, tag="acc")
            nc.vector.memset(acc[:], 0.0)

            # --- projection pass: u (local), v (all nodes) -------------------
            for w in range(nwin_full):
                hT_t = pp.tile([D + 1, WIN], bf, tag="hT_proj")
                nc.sync.dma_start(out=hT_t[:], in_=hT_full[:, w * WIN:(w + 1) * WIN])
                pv = ps3.tile([WIN, D], fp, space="PSUM", tag="mw")
                nc.tensor.matmul(pv[:], lhsT=hT_t[:], rhs=w1bT_s[:], start=True, stop=True)
                v_t = pp.tile([WIN, D], bf, tag="v_t")
                nc.vector.tensor_copy(v_t[:], pv[:])
                nc.sync.dma_start(out=v_dram[w * WIN:(w + 1) * WIN, :], in_=v_t[:])

            for w in range(NWIN):
                hT_t = pp.tile([D + 1, WIN], bf, tag="hT_proj")
                nc.sync.dma_start(out=hT_t[:], in_=hT_loc[:, w * WIN:(w + 1) * WIN])
                pu = ps3.tile([WIN, D], fp, space="PSUM", tag="mw")
                nc.tensor.matmul(pu[:], lhsT=hT_t[:], rhs=w1aT_s[:], start=True, stop=True)
                u_t = pp.tile([WIN, D], bf, tag="u_t")
                nc.vector.tensor_copy(u_t[:], pu[:])
                nc.sync.dma_start(out=u_dram[w * WIN:(w + 1) * WIN, :], in_=u_t[:])

            # --- edge pass + fused window finalize ---------------------------
            zagg = None
            for t in range(nchunk):
                n_sub_here = min(KSUB, nsub - t * KSUB)
                if n_sub_here <= 0:
                    break
                idx_t = ep.tile([SUB, 2 * KSUB], i32, tag="idx")
                nc.sync.dma_start(out=idx_t[:], in_=idx_il[t])
                slot_t = ep.tile([SUB, KSUB], fp, tag="slot")
                nc.sync.dma_start(out=slot_t[:], in_=slot_il[t])
                ea_t = ep.tile([DE, CHUNK], bf, tag="ea")
                nc.sync.dma_start(out=ea_t[:], in_=eaT[:, t * CHUNK:(t + 1) * CHUNK])

                ug = ep.tile([SUB, KSUB * D], bf, tag="ug")
                vg = ep.tile([SUB, KSUB * D], bf, tag="vg")
                for j in range(n_sub_here):
                    nc.gpsimd.indirect_dma_start(
                        out=ug[:, j * D:(j + 1) * D], out_offset=None, in_=u_dram[:],
                        in_offset=bass.IndirectOffsetOnAxis(ap=idx_t[:, j:j + 1], axis=0))
                    nc.gpsimd.indirect_dma_start(
                        out=vg[:, j * D:(j + 1) * D], out_offset=None, in_=v_dram[:],
                        in_offset=bass.IndirectOffsetOnAxis(ap=idx_t[:, KSUB + j:KSUB + j + 1], axis=0))

                s1 = ep.tile([SUB, KSUB * D], bf, tag="s1")
                nc.vector.tensor_add(s1[:, :n_sub_here * D], ug[:, :n_sub_here * D],
                                     vg[:, :n_sub_here * D])

                for j in range(n_sub_here):
                    s = t * KSUB + j  # global subtile
                    w = s // spw
                    k = s % spw

                    pw = ps3.tile([SUB, D], fp, space="PSUM", tag="mw")
                    nc.tensor.matmul(pw[:], lhsT=ea_t[:, j * SUB:(j + 1) * SUB],
                                     rhs=w1cT_s[:], start=True, stop=True)
                    pre = ep.tile([SUB, D], bf, tag="pre")
                    nc.vector.tensor_add(pre[:], s1[:, j * D:(j + 1) * D], pw[:])
                    z = ep.tile([SUB, D], bf, tag="z")
                    nc.scalar.activation(z[:], pre[:], AF.Relu)

                    B = ep.tile([SUB, SUB], bf, tag="B")
                    nc.vector.tensor_tensor(
                        out=B[:], in0=slot_t[:, j:j + 1].to_broadcast([SUB, SUB]),
                        in1=iota_f[:], op=mybir.AluOpType.is_equal)

                    if k == 0:
                        zagg = ps.tile([D, SUB], fp, space="PSUM", tag="zagg")
                    nc.tensor.matmul(zagg[:], lhsT=z[:], rhs=B[:],
                                     start=(k == 0), stop=(k == spw - 1))

                    if k == spw - 1:
                        _finalize_window(nc, wp, psw, w, zagg, locals_=dict(
                            w2T_s=w2T_s, b2row_s=b2row_s, deg_s=deg_s,
                            wihT_s=wihT_s, whhT_s=whhT_s, biasg_s=biasg_s,
                            gmT_s=gmT_s, fmT_s=fmT_s, acc=acc,
                            hT_loc=hT_loc, cT_in=cT_in, hT_out=hT_out,
                            cT_out=cT_out, gate_funcs=gate_funcs, NPC=NPC))

            # --- final partition reduction of acc ---------------------------
            pp_ = psw.tile([1, G], fp, space="PSUM", tag="pwin")
            nc.tensor.matmul(pp_[:], lhsT=ones_col[:], rhs=acc[:], start=True, stop=True)
            out_s = cp.tile([1, G], fp, tag="out_s")
            nc.vector.tensor_copy(out_s[:], pp_[:])
            nc.sync.dma_start(out=partial[:], in_=out_s[:])

    _split_dma_waits(nc)
    return nc


def _split_dma_waits(nc, max_waits=1):
    """HW instructions encode at most ~2 sync waits; spill excess waits
    onto preceding same-engine NoOps (each holding <= max_waits)."""
    for func in nc.m.functions:
        for block in func.blocks:
            insts = block.instructions
            i = 0
            while i < len(insts):
                inst = insts[i]
                si = getattr(inst, "sync_info", None)
                lim = 1
                if (si is not None and si.on_wait
                        and len(si.on_wait) > lim):
                    waits = list(si.on_wait)
                    keep = waits[:lim]
                    spill = waits[len(keep):]
                    si.on_wait = keep
                    while spill:
                        part, spill = spill[:max_waits], spill[max_waits:]
                        nop = mybir.InstNoOp(
                            name=nc.get_next_instruction_name(),
                            ins=[], outs=[],
                            sync_info=mybir.SyncInfo(on_wait=part,
                                                     on_update=[]),
                            engine=inst.engine,
                        )
                        nc.register_instruction(nop)
                        insts.insert(i, nop)
                        i += 1
                i += 1


def _finalize_window(nc, wp, psw, w, zagg, locals_):
    l = locals_
    AF = mybir.ActivationFunctionType
    fp = mybir.dt.float32
    bf = mybir.dt.bfloat16
    NPC = l["NPC"]

    zt = wp.tile([D, SUB], bf, tag="zt")
    nc.vector.tensor_copy(zt[:], zagg[:])

    pa = psw.tile([D, SUB], fp, space="PSUM", tag="pwin")
    nc.tensor.matmul(pa[:], lhsT=l["w2T_s"][:], rhs=zt[:], start=True, stop=False)
    nc.tensor.matmul(pa[:], lhsT=l["b2row_s"][:],
                     rhs=l["deg_s"][:, w * WIN:(w + 1) * WIN], start=False, stop=True)
    aT = wp.tile([D, SUB], bf, tag="aT")
    nc.vector.tensor_copy(aT[:], pa[:])

    hT_w = wp.tile([D + 1, WIN], bf, tag="hT_w")
    nc.sync.dma_start(out=hT_w[:], in_=l["hT_loc"][:, w * WIN:(w + 1) * WIN])
    cT_w = wp.tile([D, WIN], fp, tag="cT_w")
    nc.sync.dma_start(out=cT_w[:], in_=l["cT_in"][:, w * WIN:(w + 1) * WIN])

    acts = []
    for g in range(4):
        pg = psw.tile([D, SUB], fp, space="PSUM", tag="pwin")
        nc.tensor.matmul(pg[:], lhsT=l["wihT_s"][:, g * D:(g + 1) * D],
                         rhs=hT_w[0:D, :], start=True, stop=False)
        nc.tensor.matmul(pg[:], lhsT=l["whhT_s"][:, g * D:(g + 1) * D],
                         rhs=aT[:], start=False, stop=True)
        ag = wp.tile([D, SUB], fp, tag=f"act{g}")
        nc.scalar.activation(ag[:], pg[:], l["gate_funcs"][g],
                             bias=l["biasg_s"][:, g:g + 1])
        acts.append(ag)
    ai, af, agg, ao = acts

    tfc = wp.tile([D, SUB], fp, tag="tfc")
    nc.vector.tensor_mul(tfc[:], af[:], cT_w[:])
    tig = wp.tile([D, SUB], fp, tag="tig")
    nc.vector.tensor_mul(tig[:], ai[:], agg[:])
    cnew = wp.tile([D, SUB], fp, tag="cnew")
    nc.vector.tensor_add(cnew[:], tfc[:], tig[:])
    nc.sync.dma_start(out=l["cT_out"][:, w * WIN:(w + 1) * WIN], in_=cnew[:])
    tanhc = wp.tile([D, SUB], fp, tag="tanhc")
    nc.scalar.activation(tanhc[:], cnew[:], AF.Tanh)

    hnew = wp.tile([D + 1, SUB], bf, tag="hnew")
    nc.vector.tensor_mul(hnew[0:D, :], ao[:], tanhc[:])
    nc.vector.memset(hnew[D:D + 1, :], 1.0)
    nc.sync.dma_start(out=l["hT_out"][:, w * WIN:(w + 1) * WIN], in_=hnew[0:D, :])

    # readout contribution
    pgr = psw.tile([SUB, G], fp, space="PSUM", tag="pwin")
    nc.tensor.matmul(pgr[:], lhsT=hnew[:], rhs=l["gmT_s"][:], start=True, stop=True)
    gr = wp.tile([SUB, G], fp, tag="gr")
    nc.scalar.activation(gr[:], pgr[:], AF.Sigmoid)
    phv = psw.tile([SUB, G], fp, space="PSUM", tag="pwin")
    nc.tensor.matmul(phv[:], lhsT=hnew[:], rhs=l["fmT_s"][:], start=True, stop=True)
    pr = wp.tile([SUB, G], fp, tag="pr")
    nc.vector.tensor_mul(pr[:], gr[:], phv[:])

    sl = min(WIN, NPC - w * WIN)  # guard pad nodes in last window
    acc = l["acc"]
    nc.vector.tensor_add(acc[0:sl, :], acc[0:sl, :], pr[0:sl, :])


# ----------------------------------------------------------------------------
# host orchestration
# ----------------------------------------------------------------------------

def _prep_edges(cfg, edge_index, edge_attr):
    NPC, NWIN, NPC_PAD, CORES_ = cfg["NPC"], cfg["NWIN"], cfg["NPC_PAD"], cfg["CORES"]
    src = edge_index[0].astype(np.int64)
    dst = edge_index[1].astype(np.int64)
    core = dst // NPC
    ldst = dst - core * NPC
    win = ldst // WIN
    slot = ldst - win * WIN
    gsrc = (src // NPC) * NPC_PAD + (src % NPC)

    cw = core * NWIN + win
    counts = np.bincount(cw, minlength=CORES_ * NWIN)
    we = int(np.ceil(counts.max() / SUB) * SUB)
    spw = we // SUB
    ereal = NWIN * we
    nchunk = int(np.ceil(ereal / CHUNK))
    epa = nchunk * CHUNK

    order = np.argsort(cw, kind="stable")
    sorted_cw = cw[order]
    group_starts = np.searchsorted(sorted_cw, np.arange(CORES_ * NWIN))
    ranks = np.arange(len(order)) - group_starts[sorted_cw]
    wsort = sorted_cw % NWIN
    csort = sorted_cw // NWIN
    pos = wsort * we + ranks

    ne = len(order)
    slot_f = np.full((CORES_, epa), 999.0, np.float32)
    dstg = np.zeros((CORES_, epa), np.int32)
    srcg = np.zeros((CORES_, epa), np.int32)
    eaT = np.zeros((CORES_, DE, epa), BF16)

    eo = order
    slot_f[csort, pos] = slot[eo]
    dstg[csort, pos] = ldst[eo]
    srcg[csort, pos] = gsrc[eo]
    ea_bf = np.ascontiguousarray(edge_attr[eo].astype(BF16))
    eaT[csort, :, pos] = ea_bf

    def il(a):  # [epa] -> [nchunk, 128, KSUB]
        return np.ascontiguousarray(
            a.reshape(nchunk, KSUB, SUB).transpose(0, 2, 1))

    idx_il = np.zeros((CORES_, nchunk, SUB, 2 * KSUB), np.int32)
    slot_il = np.zeros((CORES_, nchunk, SUB, KSUB), np.float32)
    for c in range(CORES_):
        idx_il[c, :, :, 0:KSUB] = il(dstg[c])
        idx_il[c, :, :, KSUB:2 * KSUB] = il(srcg[c])
        slot_il[c] = il(slot_f[c])

    deg = np.bincount(core * NPC_PAD + ldst,
                      minlength=CORES_ * NPC_PAD).reshape(CORES_, NPC_PAD)
    return dict(spw=spw, nchunk=nchunk, epa=epa, eaT=eaT,
                idx_il=idx_il, slot_il=slot_il, deg=deg.astype(BF16))


def _prep_weights(i, fe_w1, fe_b1, fe_w2, fe_b2, lstm_wih, lstm_whh,
                  lstm_bih, lstm_bhh, gm_w, gm_b, fm_w, fm_b):
    w1 = np.asarray(fe_w1[i], np.float32)
    w1aT = np.vstack([w1[:, :D].T, np.asarray(fe_b1[i], np.float32)[None]])
    w1bT = np.vstack([w1[:, D:2 * D].T, np.zeros((1, D), np.float32)])
    w1cT = np.ascontiguousarray(w1[:, 2 * D:].T)
    w2T = np.asarray(fe_w2[i], np.float32).T
    b2row = np.asarray(fe_b2[i], np.float32)[None]
    wihT = np.asarray(lstm_wih[i], np.float32).T
    whhT = np.asarray(lstm_whh[i], np.float32).T
    biasg = (np.asarray(lstm_bih[i], np.float32)
             + np.asarray(lstm_bhh[i], np.float32)).reshape(4, D).T
    gmT = np.vstack([np.asarray(gm_w, np.float32).T,
                     np.asarray(gm_b, np.float32)[None]])
    fmT = np.vstack([np.asarray(fm_w, np.float32).T,
                     np.asarray(fm_b, np.float32)[None]])
    c = np.ascontiguousarray
    return dict(w1aT=c(w1aT.astype(BF16)), w1bT=c(w1bT.astype(BF16)),
                w1cT=c(w1cT.astype(BF16)), w2T=c(w2T.astype(BF16)),
                b2row=c(b2row.astype(BF16)), wihT=c(wihT.astype(BF16)),
                whhT=c(whhT.astype(BF16)), biasg=c(biasg.astype(np.float32)),
                gmT=c(gmT.astype(BF16)), fmT=c(fmT.astype(BF16)))


def _pack_hT(cfg, h_rows):
    """h_rows [N, D] float -> hT_aug [D+1, NFULL] bf16 (padded, ones row)."""
    NPC, NPC_PAD, NFULL, CORES_ = cfg["NPC"], cfg["NPC_PAD"], cfg["NFULL"], cfg["CORES"]
    out = np.zeros((D + 1, NFULL), BF16)
    out[D, :] = 1.0
    for c in range(CORES_):
        blk = h_rows[c * NPC:(c + 1) * NPC]  # [NPC, D]
        out[:D, c * NPC_PAD:c * NPC_PAD + NPC] = blk.T.astype(BF16)
    return out


def _run_model(inputs, cfg, trace=False):
    global LAST_EXEC_NS
    x = np.asarray(inputs["x"], np.float32)
    edge_attr = np.asarray(inputs["edge_attr"], np.float32)
    edge_index = np.asarray(inputs["edge_index"], np.int32)
    ep = _prep_edges(cfg, edge_index, edge_attr)
    wts = [_prep_weights(i, inputs["fe_w1"], inputs["fe_b1"], inputs["fe_w2"],
                         inputs["fe_b2"], inputs["lstm_wih"], inputs["lstm_whh"],
                         inputs["lstm_bih"], inputs["lstm_bhh"], inputs["gm_w"],
                         inputs["gm_b"], inputs["fm_w"], inputs["fm_b"])
           for i in range(P_STEPS)]

    nc = _build(cfg, ep["spw"], ep["nchunk"], ep["epa"])

    CORES_ = cfg["CORES"]
    NPC_PAD = cfg["NPC_PAD"]
    hT_full = _pack_hT(cfg, x)
    cT = [np.zeros((D, NPC_PAD), np.float32) for _ in range(CORES_)]

    total_ns = 0
    partials = None
    for step in range(P_STEPS):
        in_maps = []
        for c in range(CORES_):
            m = dict(
                hT_full=hT_full,
                hT_loc=np.ascontiguousarray(
                    hT_full[:, c * NPC_PAD:(c + 1) * NPC_PAD]),
                cT_in=cT[c],
                eaT=ep["eaT"][c],
                idx_il=ep["idx_il"][c],
                slot_il=ep["slot_il"][c],
                deg_in=ep["deg"][c][None, :],
            )
            m.update(wts[step])
            in_maps.append(m)
        import time as _time
        _t0 = _time.perf_counter()
        try:
            res = run_bass_kernel_spmd(nc, in_maps, list(range(CORES_)), trace=trace)
        except ModuleNotFoundError:
            res = run_bass_kernel_spmd(nc, in_maps, list(range(CORES_)), trace=False)
        _wall_ns = int((_time.perf_counter() - _t0) * 1e9)
        if os.environ.get("GNN_DEBUG"):
            r0 = res.results[0]
            print(f"[dbg] step{step} hT_out[:2,:3]", np.asarray(r0["hT_out"])[:2, :3])
            print(f"[dbg] step{step} cT_out[:2,:3]", np.asarray(r0["cT_out"])[:2, :3])
            print(f"[dbg] step{step} partial[:5]", np.asarray(r0["partial"])[0, :5])
        if res.exec_time_ns:
            total_ns += res.exec_time_ns
        else:
            total_ns += _wall_ns
        # reassemble h for next step
        if step < P_STEPS - 1:
            nf = cfg["NFULL"]
            hT_full = np.zeros((D + 1, nf), BF16)
            hT_full[D, :] = 1.0
            for c in range(CORES_):
                h_c = res.results[c]["hT_out"]
                hT_full[:D, c * NPC_PAD:(c + 1) * NPC_PAD] = h_c
                # zero the per-core pad columns
                hT_full[:D, c * NPC_PAD + cfg["NPC"]:(c + 1) * NPC_PAD] = 0
                cT[c] = np.ascontiguousarray(res.results[c]["cT_out"])
        else:
            partials = [res.results[c]["partial"][0] for c in range(CORES_)]

    LAST_EXEC_NS = total_ns
    out = np.sum(np.stack(partials).astype(np.float64), axis=0).astype(np.float32)
    return out


def kernel(**inputs):
    cfg = _cfg(N, E, CORES)
    trace = bool(int(os.environ.get("GNN_TRACE", "0")))
    return _run_model(inputs, cfg, trace=trace)



# revision 3
# speedup vs baseline: 6.9965x; 6.9965x over previous
"""GNN message passing (MPNN + LSTM update + gated sum pooling), 8 trn2 cores. V2.

Keyed to the dispatch-cost profile of trn2:
  - Per-edge u[dst]/v[src] projections fetched via Q7 dma_gather (994ns fixed
    + 0.34ns/row on gpsimd) in 4096-edge chunks, int16 indices. v table is
    indexed by global padded node id; edges are grouped by src-range (32768
    rows) so every chunk gathers from one table slice. u table is core-local
    (12544 rows, always in int16 range).
  - Edges ordered (src_range, dst_window); per-(r,w) edge counts padded to a
    128 multiple, uniform across cores (max), so one NEFF serves all cores.
  - Scatter-add onto the 128-node dst window via one-hot matmul in PSUM,
    drained into a persistent SBUF aggregator zagg[64, NPC_PAD] (fp32).
  - W2/b2 folded into the LSTM: gates += (whh@W2) zagg + (whh@b2) deg, so the
    message-MLP second layer never materializes.
  - h, c live in SBUF for the whole step; node pass does no DMA.
  - One step per NEFF launch; host exchanges h between the two steps.
"""

import os

import numpy as np
import ml_dtypes

import concourse.bass as bass
import concourse.mybir as mybir
import concourse.tile as tile
from concourse import library_config
from concourse.bass_utils import run_bass_kernel_spmd

BF16 = ml_dtypes.bfloat16
FP32 = np.float32

N = 100000
E = 1600000
D = 64
DE = 32
G = 50
P_STEPS = 2
CORES = 8

WIN = 128
NPC = N // CORES               # 12500
NWIN = (NPC + WIN - 1) // WIN  # 98
NPC_PAD = NWIN * WIN           # 12544
NFULL = CORES * NPC_PAD        # 100352
RANGE = 32768
NR = (NFULL + RANGE - 1) // RANGE  # 4
ES = 128                       # gather row elems (bf16) = 256B
CKSUB = 32                     # subtiles per gather chunk (4096 edges)
SCSUB = 16                     # subtiles per compute sub-chunk
PAD_SLOT = 300.0

LAST_EXEC_NS = None


def _chunk_plan(spw_rw):
    """Chunks of <= CKSUB subtiles, never spanning a range boundary.
    Returns ([(sub0, nsub)], sub_map[(r, w, k, last)])."""
    sub_map = []
    for r in range(NR):
        for w in range(NWIN):
            s = spw_rw[r][w]
            for k in range(s):
                sub_map.append((r, w, k, k == s - 1))
    chunks = []
    s0 = 0
    for r in range(NR):
        sr = sum(spw_rw[r])
        while sr > 0:
            take = min(CKSUB, sr)
            chunks.append((s0, take))
            s0 += take
            sr -= take
    return chunks, sub_map


# ----------------------------------------------------------------------------
# device kernel (one message-passing step)
# ----------------------------------------------------------------------------

def _build(spw_rw, nsub, epad):
    fp = mybir.dt.float32
    bf = mybir.dt.bfloat16
    i16 = mybir.dt.int16
    i32 = mybir.dt.int32
    AF = mybir.ActivationFunctionType

    nc = bass.Bass("TRN2", target_bir_lowering=False, debug=False,
                   num_swdge_queues=1)

    hT_loc = nc.dram_tensor("hT_loc", [D + 1, NPC_PAD], bf, kind="ExternalInput")
    hT_full = nc.dram_tensor("hT_full", [D + 1, NFULL], bf, kind="ExternalInput")
    cT_in = nc.dram_tensor("cT_in", [D, NPC_PAD], fp, kind="ExternalInput")
    ea_in = nc.dram_tensor("ea_in", [DE, epad], bf, kind="ExternalInput")
    slot_in = nc.dram_tensor("slot_in", [WIN, nsub], bf, kind="ExternalInput")
    idx_in = nc.dram_tensor("idx_in", [WIN, nsub * 16], i16, kind="ExternalInput")
    deg_in = nc.dram_tensor("deg_in", [1, NPC_PAD], bf, kind="ExternalInput")
    w1aT = nc.dram_tensor("w1aT", [D + 1, D], bf, kind="ExternalInput")
    w1bT = nc.dram_tensor("w1bT", [D + 1, D], bf, kind="ExternalInput")
    w1cT = nc.dram_tensor("w1cT", [DE, D], bf, kind="ExternalInput")
    wihT = nc.dram_tensor("wihT", [D, 4 * D], bf, kind="ExternalInput")
    whhW2T = nc.dram_tensor("whhW2T", [D, 4 * D], bf, kind="ExternalInput")
    whhb2 = nc.dram_tensor("whhb2", [1, 4 * D], bf, kind="ExternalInput")
    biasg = nc.dram_tensor("biasg", [D, 4], fp, kind="ExternalInput")
    gmT = nc.dram_tensor("gmT", [D + 1, G], bf, kind="ExternalInput")
    fmT = nc.dram_tensor("fmT", [D + 1, G], bf, kind="ExternalInput")

    hT_out = nc.dram_tensor("hT_out", [D, NPC_PAD], bf, kind="ExternalOutput")
    cT_out = nc.dram_tensor("cT_out", [D, NPC_PAD], fp, kind="ExternalOutput")
    partial = nc.dram_tensor("partial", [1, G], fp, kind="ExternalOutput")

    u_dram = nc.dram_tensor("u_dram", [NPC_PAD, ES], bf)
    v_dram = nc.dram_tensor("v_dram", [NFULL, ES], bf)

    chunks, sub_map = _chunk_plan(spw_rw)
    assert len(sub_map) == nsub

    with tile.TileContext(nc) as tc:
        with tc.tile_pool(name="const", bufs=1) as cp:
            def load_const(t, shape, dtype):
                s = cp.tile(shape, dtype, tag=t.name)
                nc.sync.dma_start(out=s[:], in_=t[:])
                return s

            w1aT_s = load_const(w1aT, [D + 1, D], bf)
            w1bT_s = load_const(w1bT, [D + 1, D], bf)
            w1cT_s = load_const(w1cT, [DE, D], bf)
            wihT_s = load_const(wihT, [D, 4 * D], bf)
            whhW2T_s = load_const(whhW2T, [D, 4 * D], bf)
            whhb2_s = load_const(whhb2, [1, 4 * D], bf)
            biasg_s = load_const(biasg, [D, 4], fp)
            gmT_s = load_const(gmT, [D + 1, G], bf)
            fmT_s = load_const(fmT, [D + 1, G], bf)

            hT_s = cp.tile([D + 1, NPC_PAD], bf, tag="hT_s")
            nc.sync.dma_start(out=hT_s[:], in_=hT_loc[:])
            cT_s = cp.tile([D, NPC_PAD], fp, tag="cT_s")
            nc.sync.dma_start(out=cT_s[:], in_=cT_in[:])
            zagg_s = cp.tile([D, NPC_PAD], fp, tag="zagg_s")
            nc.vector.memset(zagg_s[:], 0.0)

            iota_i = cp.tile([WIN, WIN], i32, tag="iota_i")
            nc.gpsimd.iota(iota_i[:], pattern=[[1, WIN]], base=0,
                           channel_multiplier=0)
            # iota runs from the default (standard) Q7 library; switch to
            # mlp for the dma_gather extended instructions used below.
            nc.gpsimd.load_library(library_config.mlp)
            iota_t = cp.tile([WIN, SCSUB * WIN], bf, tag="iota_t")
            for j in range(SCSUB):
                nc.vector.tensor_copy(iota_t[:, j * WIN:(j + 1) * WIN],
                                      iota_i[:])

            ones_col = cp.tile([WIN, 1], fp, tag="ones_col")
            nc.vector.memset(ones_col[:], 1.0)
            acc = cp.tile([WIN, G], fp, tag="acc")
            nc.vector.memset(acc[:], 0.0)

            # --- phases 1+2: u/v projections ------------------------------
            with (
                tc.tile_pool(name="proj", bufs=3) as pp,
                tc.tile_pool(name="psA", bufs=2, space="PSUM") as psA,
            ):
                uw = 0
                while uw < NWIN:
                    gn = min(8, NWIN - uw)
                    pu = psA.tile([WIN, 8 * D], fp, space="PSUM", tag="pproj")
                    for j in range(gn):
                        w = uw + j
                        nc.tensor.matmul(pu[:, j * D:(j + 1) * D],
                                         lhsT=hT_s[:, w * WIN:(w + 1) * WIN],
                                         rhs=w1aT_s[:], start=True, stop=True)
                    ut = pp.tile([WIN, 8, ES], bf, tag="u_t")
                    nc.vector.memset(ut[:, 0:gn, D:ES], 0.0)
                    nc.vector.tensor_copy(
                        ut[:, 0:gn, 0:D],
                        pu[:, 0:gn * D].rearrange("p (j d) -> p j d", d=D))
                    nc.sync.dma_start(
                        out=u_dram[uw * WIN:(uw + gn) * WIN, :]
                        .rearrange("(j p) e -> p j e", p=WIN),
                        in_=ut[:, 0:gn, :])
                    uw += gn

                NWF = NFULL // WIN  # 784
                vw = 0
                while vw < NWF:
                    gn = min(8, NWF - vw)
                    hf = pp.tile([D + 1, 8 * WIN], bf, tag="hf_t")
                    nc.sync.dma_start(out=hf[:, 0:gn * WIN],
                                      in_=hT_full[:, vw * WIN:(vw + gn) * WIN])
                    pv = psA.tile([WIN, 8 * D], fp, space="PSUM", tag="pproj")
                    for j in range(gn):
                        nc.tensor.matmul(pv[:, j * D:(j + 1) * D],
                                         lhsT=hf[:, j * WIN:(j + 1) * WIN],
                                         rhs=w1bT_s[:], start=True, stop=True)
                    vt = pp.tile([WIN, 8, ES], bf, tag="v_t")
                    nc.vector.memset(vt[:, 0:gn, D:ES], 0.0)
                    nc.vector.tensor_copy(
                        vt[:, 0:gn, 0:D],
                        pv[:, 0:gn * D].rearrange("p (j d) -> p j d", d=D))
                    nc.scalar.dma_start(
                        out=v_dram[vw * WIN:(vw + gn) * WIN, :]
                        .rearrange("(j p) e -> p j e", p=WIN),
                        in_=vt[:, 0:gn, :])
                    vw += gn

            # --- phase 3: edge pass ---------------------------------------
            with (
                tc.tile_pool(name="edge", bufs=2) as ep,
                tc.tile_pool(name="sub", bufs=2) as sp_,
                tc.tile_pool(name="psW", bufs=2, space="PSUM") as psW,
                tc.tile_pool(name="psZ", bufs=2, space="PSUM") as psZ,
            ):
                zagg_ps = None
                # gpsimd registers are a scarce pool; one per distinct
                # chunk size, reused across all gathers
                nidx_regs = {}
                for (c0, ns) in chunks:
                    r = sub_map[c0][0]
                    ck = ns * WIN
                    it = ep.tile([WIN, CKSUB * 16], i16, tag="idx")
                    nc.sync.dma_start(out=it[:, 0:ns * 16],
                                      in_=idx_in[:, c0 * 16:(c0 + ns) * 16])
                    st = ep.tile([WIN, CKSUB], bf, tag="slot")
                    nc.sync.dma_start(out=st[:, 0:ns],
                                      in_=slot_in[:, c0:c0 + ns])
                    ea_t = ep.tile([DE, CKSUB * WIN], bf, tag="ea")
                    nc.scalar.dma_start(out=ea_t[:, 0:ck],
                                        in_=ea_in[:, c0 * WIN:(c0 + ns) * WIN])

                    # Q7 gathers above ~1024 rows overflow the SWDGE ring;
                    # split into 8-subtile (1024-row) gathers.
                    ug = ep.tile([WIN, CKSUB, ES], bf, tag="ug")
                    vg = ep.tile([WIN, CKSUB, ES], bf, tag="vg")
                    rb = r * RANGE
                    q0 = 0
                    while q0 < ns:
                        qn = min(8, ns - q0)
                        qck = qn * WIN
                        if qck not in nidx_regs:
                            nidx_regs[qck] = nc.gpsimd.to_reg(qck)
                        qreg = nidx_regs[qck]
                        nc.gpsimd.dma_gather(
                            ug[:, q0:q0 + qn, :], u_dram[:],
                            it[:, q0 * 8:(q0 + qn) * 8],
                            qck, qreg, ES, queue_num=0)
                        nc.gpsimd.dma_gather(
                            vg[:, q0:q0 + qn, :],
                            v_dram[rb:min(rb + RANGE, NFULL), :],
                            it[:, ns * 8 + q0 * 8:ns * 8 + (q0 + qn) * 8],
                            qck, qreg, ES, queue_num=0)
                        q0 += qn

                    s1 = ep.tile([WIN, CKSUB * D], bf, tag="s1")
                    nc.vector.tensor_add(
                        s1[:, 0:ns * D].rearrange("p (j d) -> p j d", d=D),
                        ug[:, 0:ns, 0:D], vg[:, 0:ns, 0:D])

                    j0 = 0
                    while j0 < ns:
                        jn = min(SCSUB, ns - j0)
                        pw = psW.tile([WIN, SCSUB * D], fp, space="PSUM",
                                      tag="pw")
                        for j in range(jn):
                            nc.tensor.matmul(
                                pw[:, j * D:(j + 1) * D],
                                lhsT=ea_t[:, (j0 + j) * WIN:(j0 + j + 1) * WIN],
                                rhs=w1cT_s[:], start=True, stop=True)
                        pre = sp_.tile([WIN, SCSUB * D], bf, tag="pre")
                        nc.vector.tensor_add(pre[:, 0:jn * D],
                                             s1[:, j0 * D:(j0 + jn) * D],
                                             pw[:, 0:jn * D])
                        z = sp_.tile([WIN, SCSUB * D], bf, tag="z")
                        nc.scalar.activation(z[:, 0:jn * D], pre[:, 0:jn * D],
                                             AF.Relu)
                        B = sp_.tile([WIN, SCSUB * WIN], bf, tag="B")
                        nc.vector.tensor_tensor(
                            out=B[:, 0:jn * WIN].rearrange(
                                "p (j q) -> p j q", q=WIN),
                            in0=st[:, j0:j0 + jn].unsqueeze(2)
                            .to_broadcast([WIN, jn, WIN]),
                            in1=iota_t[:, 0:jn * WIN].rearrange(
                                "p (j q) -> p j q", q=WIN),
                            op=mybir.AluOpType.is_equal)

                        for j in range(jn):
                            s = c0 + j0 + j
                            _, w, k, last = sub_map[s]
                            if k == 0:
                                zagg_ps = psZ.tile([D, WIN], fp, space="PSUM",
                                                   tag="zagg")
                            nc.tensor.matmul(zagg_ps[:],
                                             lhsT=z[:, j * D:(j + 1) * D],
                                             rhs=B[:, j * WIN:(j + 1) * WIN],
                                             start=(k == 0), stop=last)
                            if last:
                                nc.vector.tensor_add(
                                    zagg_s[:, w * WIN:(w + 1) * WIN],
                                    zagg_s[:, w * WIN:(w + 1) * WIN],
                                    zagg_ps[:])
                        j0 += jn

            # --- phase 4: node pass (LSTM + readout), no DMA --------------
            with (
                tc.tile_pool(name="win", bufs=2) as wp,
                tc.tile_pool(name="psG", bufs=2, space="PSUM") as psG,
            ):
                deg_s = wp.tile([1, NPC_PAD], bf, tag="deg_s")
                nc.sync.dma_start(out=deg_s[:], in_=deg_in[:])
                gate_funcs = [AF.Sigmoid, AF.Sigmoid, AF.Tanh, AF.Sigmoid]
                for w in range(NWIN):
                    hw = hT_s[:, w * WIN:(w + 1) * WIN]
                    za = wp.tile([D, WIN], bf, tag="za")
                    nc.vector.tensor_copy(za[:],
                                          zagg_s[:, w * WIN:(w + 1) * WIN])

                    acts = []
                    for g in range(4):
                        pgh = psG.tile([D, WIN], fp, space="PSUM", tag="pg")
                        cols = slice(g * D, (g + 1) * D)
                        nc.tensor.matmul(pgh[:], lhsT=wihT_s[:, cols],
                                         rhs=hw[0:D, :], start=True,
                                         stop=False)
                        nc.tensor.matmul(pgh[:], lhsT=whhW2T_s[:, cols],
                                         rhs=za[:], start=False, stop=False)
                        nc.tensor.matmul(pgh[:], lhsT=whhb2_s[:, cols],
                                         rhs=deg_s[:, w * WIN:(w + 1) * WIN],
                                         start=False, stop=True)
                        ag = wp.tile([D, WIN], fp, tag=f"act{g}")
                        nc.scalar.activation(ag[:], pgh[:], gate_funcs[g],
                                             bias=biasg_s[:, g:g + 1])
                        acts.append(ag)
                    ai, af, agg_, ao = acts

                    tfc = wp.tile([D, WIN], fp, tag="tfc")
                    nc.vector.tensor_mul(tfc[:], af[:],
                                         cT_s[:, w * WIN:(w + 1) * WIN])
                    tig = wp.tile([D, WIN], fp, tag="tig")
                    nc.vector.tensor_mul(tig[:], ai[:], agg_[:])
                    nc.vector.tensor_add(cT_s[:, w * WIN:(w + 1) * WIN],
                                         tfc[:], tig[:])
                    tanhc = wp.tile([D, WIN], fp, tag="tanhc")
                    nc.scalar.activation(tanhc[:],
                                         cT_s[:, w * WIN:(w + 1) * WIN],
                                         AF.Tanh)
                    nc.vector.tensor_mul(hT_s[0:D, w * WIN:(w + 1) * WIN],
                                         ao[:], tanhc[:])

                    pgr = psG.tile([WIN, G], fp, space="PSUM", tag="pro")
                    nc.tensor.matmul(pgr[:], lhsT=hw, rhs=gmT_s[:],
                                     start=True, stop=True)
                    gr = wp.tile([WIN, G], fp, tag="gr")
                    nc.scalar.activation(gr[:], pgr[:], AF.Sigmoid)
                    phv = psG.tile([WIN, G], fp, space="PSUM", tag="pro")
                    nc.tensor.matmul(phv[:], lhsT=hw, rhs=fmT_s[:],
                                     start=True, stop=True)
                    pr = wp.tile([WIN, G], fp, tag="pr")
                    nc.vector.tensor_mul(pr[:], gr[:], phv[:])
                    sl = min(WIN, NPC - w * WIN)
                    nc.vector.tensor_add(acc[0:sl, :], acc[0:sl, :],
                                         pr[0:sl, :])

                # --- outputs --------------------------------------------
                nc.sync.dma_start(out=hT_out[:], in_=hT_s[0:D, :])
                nc.sync.dma_start(out=cT_out[:], in_=cT_s[:])
                pfin = psG.tile([1, G], fp, space="PSUM", tag="pfin")
                nc.tensor.matmul(pfin[:], lhsT=ones_col[:], rhs=acc[:],
                                 start=True, stop=True)
                out_s = cp.tile([1, G], fp, tag="out_s")
                nc.vector.tensor_copy(out_s[:], pfin[:])
                nc.sync.dma_start(out=partial[:], in_=out_s[:])

    _split_dma_waits(nc)
    # raw Bass skips codegen_inst_isa_subclasses (Bacc runs it); without it
    # the pseudo reload-library instruction has empty .instr bytes and
    # walrus fails with "ISA wrong length".
    mybir.codegen_inst_isa_subclasses(nc)
    return nc


def _split_dma_waits(nc, max_waits=1):
    """Walrus encodes at most ~2 sem waits per instruction; spill extras
    onto same-engine NoOps."""
    for func in nc.m.functions:
        for block in func.blocks:
            insts = block.instructions
            i = 0
            while i < len(insts):
                inst = insts[i]
                si = getattr(inst, "sync_info", None)
                if si is not None and si.on_wait and len(si.on_wait) > 1:
                    waits = list(si.on_wait)
                    si.on_wait = waits[:1]
                    spill = waits[1:]
                    while spill:
                        part, spill = spill[:max_waits], spill[max_waits:]
                        nop = mybir.InstNoOp(
                            name=nc.get_next_instruction_name(),
                            ins=[], outs=[],
                            sync_info=mybir.SyncInfo(on_wait=part,
                                                     on_update=[]),
                            engine=inst.engine,
                        )
                        nc.register_instruction(nop)
                        insts.insert(i, nop)
                        i += 1
                i += 1


# ----------------------------------------------------------------------------
# host orchestration
# ----------------------------------------------------------------------------

def _prep_edges(edge_index, edge_attr):
    src = edge_index[0].astype(np.int64)
    dst = edge_index[1].astype(np.int64)
    core = dst // NPC
    ldst = dst - core * NPC
    w = ldst // WIN
    slot = ldst - w * WIN
    gsrc = (src // NPC) * NPC_PAD + (src % NPC)
    r = gsrc // RANGE

    key = (core * NR + r) * NWIN + w
    cnt = np.bincount(key, minlength=CORES * NR * NWIN).reshape(
        CORES, NR, NWIN)
    spw_rw = np.ceil(cnt.max(axis=0) / WIN).astype(np.int64)
    nsub = int(spw_rw.sum())
    epad = nsub * WIN

    base = np.zeros((NR, NWIN), np.int64)
    acc_ = 0
    for rr in range(NR):
        for ww in range(NWIN):
            base[rr, ww] = acc_
            acc_ += spw_rw[rr, ww] * WIN

    order = np.argsort(key, kind="stable")
    sorted_key = key[order]
    group_starts = np.searchsorted(sorted_key, np.arange(CORES * NR * NWIN))
    ranks = np.arange(len(order)) - group_starts[sorted_key]
    pos = base[r[order], w[order]] + ranks
    csort = core[order]

    ea_s = np.zeros((CORES, DE, epad), BF16)
    slot_flat = np.full((CORES, epad), PAD_SLOT, np.float32)
    uidx = np.zeros((CORES, epad), np.int16)
    vidx = np.zeros((CORES, epad), np.int16)

    eo = order
    ea_s[csort, :, pos] = edge_attr[eo].astype(BF16)
    slot_flat[csort, pos] = slot[eo]
    uidx[csort, pos] = ldst[eo].astype(np.int16)
    vidx[csort, pos] = (gsrc[eo] - r[eo] * RANGE).astype(np.int16)

    # slot per subtile: [128, nsub], [p, s] = slot of edge s*128+p
    slot_s = np.ascontiguousarray(
        slot_flat.reshape(CORES, nsub, WIN).transpose(0, 2, 1)).astype(BF16)

    # idx stream: per chunk, u-wrapped block then v-wrapped block.
    # wrapped: within a chunk of ck edges, index i at [i%16, i//16].
    chunks, _ = _chunk_plan(spw_rw.tolist())
    idx_pack = np.zeros((CORES, 16, nsub * 16), np.int16)

    def wrap(a):  # [CORES, ck] -> [CORES, 16, ck//16]
        ck = a.shape[1]
        return a.reshape(CORES, ck // 16, 16).transpose(0, 2, 1)

    for (c0, ns) in chunks:
        e0, ck = c0 * WIN, ns * WIN
        col = c0 * 16
        idx_pack[:, :, col:col + ns * 8] = wrap(uidx[:, e0:e0 + ck])
        idx_pack[:, :, col + ns * 8:col + ns * 16] = wrap(vidx[:, e0:e0 + ck])

    deg = np.bincount(core * NPC_PAD + ldst,
                      minlength=CORES * NPC_PAD).reshape(CORES, NPC_PAD)
    # Q7 cores each read their own 16-partition group: replicate 8x
    idx_rep = np.ascontiguousarray(np.tile(idx_pack, (1, 8, 1)))
    return dict(spw_rw=spw_rw.tolist(), nsub=nsub, epad=epad, ea=ea_s,
                slot=slot_s, idx=idx_rep, deg=deg.astype(BF16))


def _prep_weights(i, fe_w1, fe_b1, fe_w2, fe_b2, lstm_wih, lstm_whh,
                  lstm_bih, lstm_bhh, gm_w, gm_b, fm_w, fm_b):
    w1 = np.asarray(fe_w1[i], FP32)
    w1aT = np.vstack([w1[:, :D].T, np.asarray(fe_b1[i], FP32)[None]])
    w1bT = np.vstack([w1[:, D:2 * D].T, np.zeros((1, D), FP32)])
    w1cT = np.ascontiguousarray(w1[:, 2 * D:].T)
    whh = np.asarray(lstm_whh[i], FP32)
    w2 = np.asarray(fe_w2[i], FP32)
    b2 = np.asarray(fe_b2[i], FP32)
    whhW2 = whh @ w2
    whhb2 = (whh @ b2)[None]
    wihT = np.asarray(lstm_wih[i], FP32).T
    bias = np.asarray(lstm_bih[i], FP32) + np.asarray(lstm_bhh[i], FP32)
    biasg = np.ascontiguousarray(bias.reshape(4, D).T)
    gmT = np.vstack([np.asarray(gm_w, FP32).T, np.asarray(gm_b, FP32)[None]])
    fmT = np.vstack([np.asarray(fm_w, FP32).T, np.asarray(fm_b, FP32)[None]])
    c = np.ascontiguousarray
    return dict(w1aT=c(w1aT.astype(BF16)), w1bT=c(w1bT.astype(BF16)),
                w1cT=c(w1cT.astype(BF16)), wihT=c(wihT.astype(BF16)),
                whhW2T=c(whhW2.T.astype(BF16)), whhb2=c(whhb2.astype(BF16)),
                biasg=c(biasg.astype(FP32)), gmT=c(gmT.astype(BF16)),
                fmT=c(fmT.astype(BF16)))


def _pack_hT_full(h_rows):
    out = np.zeros((D + 1, NFULL), BF16)
    for c in range(CORES):
        blk = h_rows[c * NPC:(c + 1) * NPC]
        out[:D, c * NPC_PAD:c * NPC_PAD + NPC] = blk.T.astype(BF16)
        out[D, c * NPC_PAD:c * NPC_PAD + NPC] = 1.0
    return out


def _run_model(inputs, trace=False):
    global LAST_EXEC_NS
    x = np.asarray(inputs["x"], FP32)
    edge_attr = np.asarray(inputs["edge_attr"], FP32)
    edge_index = np.asarray(inputs["edge_index"], np.int32)
    ep = _prep_edges(edge_index, edge_attr)
    wts = [_prep_weights(i, inputs["fe_w1"], inputs["fe_b1"], inputs["fe_w2"],
                         inputs["fe_b2"], inputs["lstm_wih"],
                         inputs["lstm_whh"], inputs["lstm_bih"],
                         inputs["lstm_bhh"], inputs["gm_w"], inputs["gm_b"],
                         inputs["fm_w"], inputs["fm_b"])
           for i in range(P_STEPS)]

    nc = _build(ep["spw_rw"], ep["nsub"], ep["epad"])

    hT_full = _pack_hT_full(x)
    cT = [np.zeros((D, NPC_PAD), FP32) for _ in range(CORES)]

    total_ns = 0
    partials = None
    import time as _time
    for step in range(P_STEPS):
        in_maps = []
        for c in range(CORES):
            m = dict(
                hT_loc=np.ascontiguousarray(
                    hT_full[:, c * NPC_PAD:(c + 1) * NPC_PAD]),
                hT_full=hT_full,
                cT_in=cT[c],
                ea_in=ep["ea"][c],
                slot_in=ep["slot"][c],
                idx_in=ep["idx"][c],
                deg_in=ep["deg"][c][None, :],
            )
            m.update(wts[step])
            in_maps.append(m)
        _t0 = _time.perf_counter()
        try:
            res = run_bass_kernel_spmd(nc, in_maps, list(range(CORES)),
                                       trace=trace)
        except ModuleNotFoundError:
            res = run_bass_kernel_spmd(nc, in_maps, list(range(CORES)),
                                       trace=False)
        except Exception:
            # transient INTERNAL launch failures happen on this stack;
            # one retry without tracing
            res = run_bass_kernel_spmd(nc, in_maps, list(range(CORES)),
                                       trace=False)
        _wall_ns = int((_time.perf_counter() - _t0) * 1e9)
        total_ns += res.exec_time_ns if res.exec_time_ns else _wall_ns
        if step < P_STEPS - 1:
            hT_full = np.zeros((D + 1, NFULL), BF16)
            for c in range(CORES):
                hT_full[:D, c * NPC_PAD:(c + 1) * NPC_PAD] = \
                    res.results[c]["hT_out"]
                hT_full[:D, c * NPC_PAD + NPC:(c + 1) * NPC_PAD] = 0
                hT_full[D, c * NPC_PAD:c * NPC_PAD + NPC] = 1.0
                cT[c] = np.ascontiguousarray(res.results[c]["cT_out"])
        else:
            partials = [res.results[c]["partial"][0] for c in range(CORES)]

    LAST_EXEC_NS = total_ns
    return np.sum(np.stack(partials).astype(np.float64), axis=0).astype(FP32)


def kernel(**inputs):
    trace = bool(int(os.environ.get("GNN_TRACE", "0")))
    return _run_model(inputs, trace=trace)
